# revision 1
# baseline (speedup 1.0000x reference)
"""Trainium2 Bass kernel for nn_AttentionHeads (PaiNN-style GNN edge attention).

Computes, per edge e with endpoints (i, j) = nbrs[e]:
    q = W_q @ x_i[i]; k = W_k @ x_i[j]           (per-head linears)
    dk = silu(W_dk @ feats(dist[e]))              (RBF * cosine envelope)
    weights[e, h] = silu(sum_f q*k*dk)

Strategy (8 NeuronCores, data-parallel over edges):
  - x table stored in SBUF as fp16 rows [x | x] (256B); xi / xj fetched with
    SBUF-source dma_gather(transpose=True), landing directly in the
    [feature_partition, edge_free] layout the TensorEngine needs.
  - RBF features come from a host-precomputed table over 16384 distance bins
    (pure function of the bin grid), also gathered transposed; the 20 feature
    rows are replicated at partition offsets 0/32/64/96 so the four W_dk
    channel-chunk matmuls can row-tile.
  - Per 512-edge group: Q/K matmuls row-tiled (rows 0-63 / 64-127), dkpre
    matmuls at 32-row strips, z=q*k (DVE), dk=silu(dkpre) (ACT, bias folds
    b_dk), p=z*dk (DVE fp16 2x), head-reduction via mask matmuls accumulated
    in PSUM, final silu (ACT) into an fp16 [8, E] output buffer.
"""

import numpy as np

N_NODES = 20000
N_EDGES = 150000
FEAT = 64
HEADS = 8
N_RBF = 20
CUTOFF = 5.0

N_CORES = 8
GROUP = 512                    # edges per compute group
NGROUP = 37                    # groups per core
EC = GROUP * NGROUP            # padded edges per core = 18944
E_BASE = N_EDGES // N_CORES    # real edges per core = 18750
NBINS = 16384                  # distance bins for the feats table
NODE_PAD = 20096               # nodes padded to multiple of 128 (157 ranks)
CH = 4                         # channel chunks of 128 (= 2 heads each)
QUARTER_GROUPS = (10, 9, 9, 9)  # gather pipelining split
ACT_FN = "Silu"  # overridden to Sigmoid by the CoreSim test (sim lacks Silu)
COPY_PATTERN = "mix"  # qk PSUM->SBUF drain engine: "mix" | "act" | "dve"


def _silu(v):
    return v / (1.0 + np.exp(-v))


def _feats_of(d):
    # [len(d), N_RBF] float64: sin(n*pi*d/cutoff)/d * cosine envelope
    n = np.arange(1, N_RBF + 1, dtype=np.float64)
    s = np.sin(n * np.pi * d[:, None] / CUTOFF) / d[:, None]
    env = np.where(d < CUTOFF, 0.5 * (np.cos(np.pi * d / CUTOFF) + 1.0), 0.0)
    return s * env[:, None]


def _wrap_idx(idx):
    # dma_gather index layout: position i -> partition i%16, free i//16,
    # replicated across the 8 Q7 core groups (128 partitions total).
    n = idx.shape[0]
    w = idx.reshape(n // 16, 16).T.astype(np.int16)  # [16, n//16]
    return np.ascontiguousarray(np.tile(w, (8, 1)))  # [128, n//16]


def _table_sbuf_layout(rows):
    # rows: [n_tokens, 128] (n_tokens multiple of 128). SBUF-source gather
    # reads token t of rank r from partition t, free bytes [r*256, r*256+256).
    n = rows.shape[0]
    assert n % 128 == 0 and rows.shape[1] == 128
    return np.ascontiguousarray(
        rows.reshape(n // 128, 128, 128).transpose(1, 0, 2).reshape(128, n)
    )


_PROGRAM_CACHE = {}


def _build_program(with_qk_bias):
    import concourse.tile as tile
    from concourse import bacc, mybir

    key = (bool(with_qk_bias), ACT_FN, COPY_PATTERN, EC)
    if key in _PROGRAM_CACHE:
        return _PROGRAM_CACHE[key]

    f16 = mybir.dt.float16
    f32 = mybir.dt.float32
    i16 = mybir.dt.int16
    AF = mybir.ActivationFunctionType
    AF_FN = getattr(AF, ACT_FN)

    nc = bacc.Bacc("TRN2", target_bir_lowering=False, debug=False)

    xtab_d = nc.dram_tensor("xtab", [128, NODE_PAD], f16, kind="ExternalInput")
    ftab_d = nc.dram_tensor("ftab", [128, NBINS], f16, kind="ExternalInput")
    wqk_d = nc.dram_tensor("wqk", [128, 512], f16, kind="ExternalInput")
    wdk_d = nc.dram_tensor("wdk", [128, 512], f16, kind="ExternalInput")
    mask_d = nc.dram_tensor("mask4", [128, 32], f16, kind="ExternalInput")
    bdk_d = nc.dram_tensor("bdk", [128, 4], f32, kind="ExternalInput")
    idxi_d = nc.dram_tensor("idxi", [128, EC // 16], i16, kind="ExternalInput")
    idxj_d = nc.dram_tensor("idxj", [128, EC // 16], i16, kind="ExternalInput")
    idxb_d = nc.dram_tensor("idxb", [128, EC // 16], i16, kind="ExternalInput")
    if with_qk_bias:
        bqk_d = nc.dram_tensor("bqk", [128, 8], f32, kind="ExternalInput")
    wout_d = nc.dram_tensor("wout", [8, EC], f16, kind="ExternalOutput")

    with tile.TileContext(nc) as tc:
        with (
            tc.tile_pool(name="tabs", bufs=1) as tabs,
            tc.tile_pool(name="gath", bufs=2) as gath,
            tc.tile_pool(name="work", bufs=4) as work,
            tc.tile_pool(name="outp", bufs=1) as outp,
            tc.tile_pool(name="psum", bufs=2, space="PSUM") as psum,
        ):
            # ---- resident tables / constants ----
            xtab = tabs.tile([128, NODE_PAD], f16)
            ftab = tabs.tile([128, NBINS], f16)
            wqk = tabs.tile([128, 512], f16)
            wdk = tabs.tile([128, 512], f16)
            mask4 = tabs.tile([128, 32], f16)
            bdk = tabs.tile([128, 4], f32)
            idxi = tabs.tile([128, EC // 16], i16)
            idxj = tabs.tile([128, EC // 16], i16)
            idxb = tabs.tile([128, EC // 16], i16)
            w_all = outp.tile([8, EC], f16)

            # x-table + its idx lists first: the xi/xj gathers depend only
            # on these, so they can start while the feats table still loads
            nc.sync.dma_start(xtab[:], xtab_d[:])
            nc.sync.dma_start(idxi[:], idxi_d[:])
            nc.sync.dma_start(idxj[:], idxj_d[:])
            nc.sync.dma_start(wqk[:], wqk_d[:])
            nc.sync.dma_start(wdk[:], wdk_d[:])
            nc.sync.dma_start(mask4[:], mask_d[:])
            nc.sync.dma_start(bdk[:], bdk_d[:])
            nc.sync.dma_start(ftab[:], ftab_d[:])
            nc.sync.dma_start(idxb[:], idxb_d[:])
            if with_qk_bias:
                bqk = tabs.tile([128, 8], f32)
                nc.sync.dma_start(bqk[:], bqk_d[:])

            GCHUNK = 2048  # idxs per dma_gather call (ring capacity limit)

            def one_gather(dst, table, idx, idx_col0, o, m):
                nc.gpsimd.dma_gather(
                    dst[:, :, o : o + m],
                    table[:],
                    idx[:, idx_col0 + o // 16 : idx_col0 + (o + m) // 16],
                    num_idxs=m,
                    num_idxs_reg=m,
                    elem_size=128,
                    transpose=True,
                    sbuf_tokens_per_rank=128,
                    sbuf_free_dim_per_rank=256,
                    sbuf_free_dim_pad_per_rank=0,
                    sbuf_byte_offset=0,
                    single_packet=False,
                )

            g0 = 0
            for qi, qgroups in enumerate(QUARTER_GROUPS):
                qe = qgroups * GROUP          # edges this quarter
                e0 = g0 * GROUP               # first edge of quarter
                c0, c1 = e0 // 16, (e0 + qe) // 16
                xiT = gath.tile([128, 1, QUARTER_GROUPS[0] * GROUP], f16, tag="xiT")
                xjT = gath.tile([128, 1, QUARTER_GROUPS[0] * GROUP], f16, tag="xjT")
                ftT = gath.tile([128, 1, QUARTER_GROUPS[0] * GROUP], f16, tag="ftT")
                for tdst, ttab, tidx in (
                    (xiT, xtab, idxi), (xjT, xtab, idxj), (ftT, ftab, idxb)
                ):
                    for o in range(0, qe, GCHUNK):
                        m = min(GCHUNK, qe - o)
                        one_gather(tdst, ttab, tidx, c0, o, m)

                for g in range(qgroups):
                    gg = g0 + g               # global group index
                    s = g * GROUP             # edge offset within quarter
                    w_ps = psum.tile([8, GROUP], f32, tag="w")
                    for c in range(CH):
                        qk_ps = psum.tile([128, 2 * GROUP], f32, tag="qk")
                        d_ps = psum.tile([128, GROUP], f32, tag="d")
                        cs = slice(c * 128, (c + 1) * 128)
                        nc.tensor.matmul(
                            qk_ps[:, 0:GROUP],
                            wqk[0:64, cs],
                            xiT[0:64, 0, s : s + GROUP],
                        )
                        nc.tensor.matmul(
                            qk_ps[:, GROUP : 2 * GROUP],
                            wqk[64:128, cs],
                            xjT[64:128, 0, s : s + GROUP],
                        )
                        nc.tensor.matmul(
                            d_ps[:],
                            wdk[32 * c : 32 * c + N_RBF, cs],
                            ftT[32 * c : 32 * c + N_RBF, 0, s : s + GROUP],
                            tile_position=(32 * c, 0),
                        )
                        if with_qk_bias:
                            nc.vector.tensor_scalar_add(
                                qk_ps[:, 0:GROUP], qk_ps[:, 0:GROUP],
                                bqk[:, c : c + 1],
                            )
                            nc.vector.tensor_scalar_add(
                                qk_ps[:, GROUP : 2 * GROUP],
                                qk_ps[:, GROUP : 2 * GROUP],
                                bqk[:, 4 + c : 5 + c],
                            )
                        kc_sb = work.tile([128, GROUP], f32, tag="kc")
                        z_sb = work.tile([128, GROUP], f16, tag="z")
                        dk_sb = work.tile([128, GROUP], f16, tag="dk")
                        p_sb = work.tile([128, GROUP], f16, tag="p")
                        # drain k PSUM->SBUF; mostly ACT (DVE is the busier
                        # engine: z at PSUM-1x + p)
                        use_act = (gg * CH + c) % 3 != 2 if COPY_PATTERN == "mix" \
                            else COPY_PATTERN == "act"
                        if use_act:
                            nc.scalar.copy(kc_sb[:], qk_ps[:, GROUP : 2 * GROUP])
                        else:
                            nc.vector.tensor_copy(
                                kc_sb[:], qk_ps[:, GROUP : 2 * GROUP]
                            )
                        nc.vector.tensor_mul(
                            z_sb[:], qk_ps[:, 0:GROUP], kc_sb[:]
                        )
                        nc.scalar.activation(
                            dk_sb[:], d_ps[:], AF_FN, bias=bdk[:, c : c + 1]
                        )
                        nc.vector.tensor_mul(p_sb[:], z_sb[:], dk_sb[:])
                        nc.tensor.matmul(
                            w_ps[:],
                            mask4[:, 8 * c : 8 * c + 8],
                            p_sb[:],
                            start=(c == 0),
                            stop=(c == CH - 1),
                            skip_group_check=True,
                        )
                    nc.scalar.activation(
                        w_all[:, gg * GROUP : (gg + 1) * GROUP], w_ps[:], AF_FN
                    )
                g0 += qgroups

            nc.sync.dma_start(wout_d[:], w_all[:])

    nc.compile()
    _PROGRAM_CACHE[key] = nc
    return nc


def _prep_inputs(dist, nbrs, x_i, W_q, b_q, W_k, b_k, W_dk, b_dk):
    f16 = np.float16
    # x table: fp16 rows [x | x], padded to NODE_PAD tokens
    xd = np.zeros((NODE_PAD, 128), f16)
    xh = x_i.astype(f16)
    xd[:N_NODES, :64] = xh
    xd[:N_NODES, 64:] = xh
    xtab = _table_sbuf_layout(xd)

    # feats table over NBINS distance bins, 20 cols replicated at 0/32/64/96
    hbin = (CUTOFF - 0.5) / (NBINS - 1)
    dgrid = 0.5 + hbin * np.arange(NBINS)
    fg = _feats_of(dgrid).astype(f16)  # [NBINS, 20]
    fd = np.zeros((NBINS, 128), f16)
    for s in range(4):
        fd[:, 32 * s : 32 * s + N_RBF] = fg
    ftab = _table_sbuf_layout(fd)

    # weights in lhsT layout [f_in, h*64+g]
    wqk = np.zeros((128, 512), f16)
    wqk[:64] = W_q.transpose(2, 0, 1).reshape(64, 512).astype(f16)
    wqk[64:] = W_k.transpose(2, 0, 1).reshape(64, 512).astype(f16)
    wdk_full = W_dk.transpose(2, 0, 1).reshape(N_RBF, 512).astype(f16)
    wdk = np.zeros((128, 512), f16)
    for c in range(CH):
        wdk[32 * c : 32 * c + N_RBF, 128 * c : 128 * (c + 1)] = wdk_full[
            :, 128 * c : 128 * (c + 1)
        ]

    # head-reduction masks: chunk c covers heads 2c (rows 0-63), 2c+1 (64-127)
    mask4 = np.zeros((128, 32), f16)
    for c in range(CH):
        mask4[0:64, 8 * c + 2 * c] = 1.0
        mask4[64:128, 8 * c + 2 * c + 1] = 1.0

    bdk = np.zeros((128, 4), np.float32)
    for c in range(CH):
        bdk[0:64, c] = b_dk[2 * c]
        bdk[64:128, c] = b_dk[2 * c + 1]

    with_qk_bias = bool(np.any(b_q) or np.any(b_k))
    bqk = None
    if with_qk_bias:
        bqk = np.zeros((128, 8), np.float32)
        for c in range(CH):
            bqk[0:64, c] = b_q[2 * c]
            bqk[64:128, c] = b_q[2 * c + 1]
            bqk[0:64, 4 + c] = b_k[2 * c]
            bqk[64:128, 4 + c] = b_k[2 * c + 1]

    hb = (CUTOFF - 0.5) / (NBINS - 1)
    bins_all = np.clip(np.round((dist - 0.5) / hb), 0, NBINS - 1).astype(np.int64)

    in_maps = []
    for c in range(N_CORES):
        lo = c * E_BASE
        ii = np.zeros(EC, np.int64)
        jj = np.zeros(EC, np.int64)
        bb = np.zeros(EC, np.int64)
        ii[:E_BASE] = nbrs[lo : lo + E_BASE, 0]
        jj[:E_BASE] = nbrs[lo : lo + E_BASE, 1]
        bb[:E_BASE] = bins_all[lo : lo + E_BASE]
        m = {
            "xtab": xtab,
            "ftab": ftab,
            "wqk": wqk,
            "wdk": wdk,
            "mask4": mask4,
            "bdk": bdk,
            "idxi": _wrap_idx(ii),
            "idxj": _wrap_idx(jj),
            "idxb": _wrap_idx(bb),
        }
        if with_qk_bias:
            m["bqk"] = bqk
        in_maps.append(m)
    return in_maps, with_qk_bias


def kernel(dist, nbrs, x_i, W_q, b_q, W_k, b_k, W_dk, b_dk):
    from concourse.bass_utils import run_bass_kernel_spmd

    in_maps, with_qk_bias = _prep_inputs(
        np.asarray(dist), np.asarray(nbrs), np.asarray(x_i),
        np.asarray(W_q), np.asarray(b_q), np.asarray(W_k), np.asarray(b_k),
        np.asarray(W_dk), np.asarray(b_dk),
    )
    nc = _build_program(with_qk_bias)
    res = run_bass_kernel_spmd(nc, in_maps, list(range(N_CORES))).results

    out = np.empty((N_EDGES, HEADS), np.float32)
    for c in range(N_CORES):
        w = res[c]["wout"]  # [8, EC] fp16
        out[c * E_BASE : (c + 1) * E_BASE] = w[:, :E_BASE].T.astype(np.float32)
    return out



# revision 8
# speedup vs baseline: 1.4114x; 1.4114x over previous
"""Trainium2 Bass kernel for nn_AttentionHeads (PaiNN-style GNN edge attention).

Computes, per edge e with endpoints (i, j) = nbrs[e]:
    q = W_q @ x_i[i]; k = W_k @ x_i[j]           (per-head linears)
    dk = silu(W_dk @ feats(dist[e]))              (RBF * cosine envelope)
    weights[e, h] = silu(sum_f q*k*dk)

Strategy (8 NeuronCores, data-parallel over edges):
  - dk is a pure function of the binned distance, so the whole
    silu(W_dk @ feats + b_dk) vector (512 fp16 = 1KB per bin) is precomputed
    on the host over 16384 bins and fetched per edge with a single
    dma_gather(transpose=True) straight from HBM — no on-chip RBF matmul or
    silu.  xi / xj are gathered the same way from 256B [x | x] rows.
  - Per 512-edge group, 4 channel chunks (2 heads each): q and k matmuls into
    separate PSUM banks, k drained to fp16 SBUF by the Activation engine
    (the HW allows only one PSUM operand per vector op), z = q*kc on DVE
    (one chunk per group on the Pool engine instead), p = z*dk at the DVE
    16-bit 2x rate.
  - The head-reduction mask matmuls + final silu for group g are issued
    during group g+1 (one-group software pipeline lag) so no engine waits
    on the elementwise chain.
  - Gathers run in 2048-edge windows, two windows ahead of compute, one
    issue per group to keep the Pool engine's work smooth.
"""

import numpy as np

N_NODES = 20000
N_EDGES = 150000
FEAT = 64
HEADS = 8
N_RBF = 20
CUTOFF = 5.0

N_CORES = 8
GROUP = 512                    # edges per compute group
NGROUP = 37                    # groups per core
EC = GROUP * NGROUP            # padded edges per core = 18944
E_BASE = N_EDGES // N_CORES    # real edges per core = 18750
NBINS = 16384                  # distance bins for the dk table
NODE_PAD = 20096               # nodes padded to multiple of 128
CH = 4                         # channel chunks of 128 (= 2 heads each)
WINDOW = 2048                  # edges per gather window
ACT_FN = "Silu"
# chunk indices whose p-mul runs on the Pool engine (GPSIMD cannot read
# PSUM, so only the all-SBUF p-mul can be offloaded), by group parity
POOL_P = ((1, 2), (1, 2, 3))


def _silu(v):
    return v / (1.0 + np.exp(-v))


def _feats_of(d):
    # [len(d), N_RBF] float64: sin(n*pi*d/cutoff)/d * cosine envelope
    n = np.arange(1, N_RBF + 1, dtype=np.float64)
    s = np.sin(n * np.pi * d[:, None] / CUTOFF) / d[:, None]
    env = np.where(d < CUTOFF, 0.5 * (np.cos(np.pi * d / CUTOFF) + 1.0), 0.0)
    return s * env[:, None]


def _wrap_idx(idx):
    # dma_gather index layout: position i -> partition i%16, free i//16,
    # replicated across the 8 Q7 core groups (128 partitions total).
    n = idx.shape[0]
    w = idx.reshape(n // 16, 16).T.astype(np.int16)  # [16, n//16]
    return np.ascontiguousarray(np.tile(w, (8, 1)))  # [128, n//16]


_PROGRAM_CACHE = {}


def _build_program(with_qk_bias):
    import concourse.tile as tile
    from concourse import bacc, mybir

    key = (bool(with_qk_bias), ACT_FN, EC, POOL_P)
    if key in _PROGRAM_CACHE:
        return _PROGRAM_CACHE[key]

    f16 = mybir.dt.float16
    f32 = mybir.dt.float32
    i16 = mybir.dt.int16
    AF = mybir.ActivationFunctionType
    AF_FN = getattr(AF, ACT_FN)

    nc = bacc.Bacc("TRN2", target_bir_lowering=False, debug=False)

    xtab_d = nc.dram_tensor("xtab", [NODE_PAD, 128], f16, kind="ExternalInput")
    dktab_d = nc.dram_tensor("dktab", [NBINS, 512], f16, kind="ExternalInput")
    wqk_d = nc.dram_tensor("wqk", [128, 512], f16, kind="ExternalInput")
    mask_d = nc.dram_tensor("mask4", [128, 32], f16, kind="ExternalInput")
    idxi_d = nc.dram_tensor("idxi", [128, EC // 16], i16, kind="ExternalInput")
    idxj_d = nc.dram_tensor("idxj", [128, EC // 16], i16, kind="ExternalInput")
    idxb_d = nc.dram_tensor("idxb", [128, EC // 16], i16, kind="ExternalInput")
    if with_qk_bias:
        bqk_d = nc.dram_tensor("bqk", [128, 8], f32, kind="ExternalInput")
    wout_d = nc.dram_tensor("wout", [8, EC], f16, kind="ExternalOutput")

    wins = []
    o = 0
    while o < EC:
        wins.append((o, min(WINDOW, EC - o)))
        o += WINDOW

    with tile.TileContext(nc) as tc:
        with (
            tc.tile_pool(name="tabs", bufs=1) as tabs,
            tc.tile_pool(name="gath", bufs=3) as gath,
            tc.tile_pool(name="work", bufs=3) as work,
            tc.tile_pool(name="pgrp", bufs=2) as pgrp,
            tc.tile_pool(name="outp", bufs=1) as outp,
            tc.tile_pool(name="psum_q", bufs=3, space="PSUM") as psum_q,
            tc.tile_pool(name="psum_k", bufs=3, space="PSUM") as psum_k,
            tc.tile_pool(name="psum_w", bufs=2, space="PSUM") as psum_w,
        ):
            # ---- resident constants ----
            wqk = tabs.tile([128, 512], f16)
            mask4 = tabs.tile([128, 32], f16)
            idxi = tabs.tile([128, EC // 16], i16)
            idxj = tabs.tile([128, EC // 16], i16)
            idxb = tabs.tile([128, EC // 16], i16)
            w_all = outp.tile([8, EC], f16)

            # index lists first: gathers depend only on these
            nc.sync.dma_start(idxi[:], idxi_d[:])
            nc.sync.dma_start(idxj[:], idxj_d[:])
            nc.sync.dma_start(idxb[:], idxb_d[:])
            nc.sync.dma_start(wqk[:], wqk_d[:])
            nc.sync.dma_start(mask4[:], mask_d[:])
            if with_qk_bias:
                bqk = tabs.tile([128, 8], f32)
                nc.sync.dma_start(bqk[:], bqk_d[:])

            wtiles = {}

            def window_calls(w):
                # allocate window w's tiles, return its gather-call thunks
                o0, m = wins[w]
                xiT = gath.tile([128, 1, WINDOW], f16, tag="xi")
                xjT = gath.tile([128, 1, WINDOW], f16, tag="xj")
                if m == WINDOW:
                    dkT = gath.tile([128, CH, WINDOW], f16, tag="dk")
                else:
                    dkT = gath.tile([128, CH, m], f16, tag=f"dk{m}")
                wtiles[w] = (xiT, xjT, dkT)
                c0 = o0 // 16
                c1 = (o0 + m) // 16
                calls = []
                for tdst, tab, tidx, esz in (
                    (xiT, xtab_d, idxi, 128),
                    (xjT, xtab_d, idxj, 128),
                    (dkT, dktab_d, idxb, 512),
                ):
                    calls.append((tdst, tab, tidx, c0, c1, m, esz))
                return calls

            def one_gather(tdst, tab, tidx, c0, c1, m, esz):
                nc.gpsimd.dma_gather(
                    tdst[:, :, 0:m],
                    tab[:],
                    tidx[:, c0:c1],
                    num_idxs=m,
                    num_idxs_reg=m,
                    elem_size=esz,
                    transpose=True,
                    single_packet=False,
                )

            # pipeline state: mask matmuls + final silu for group `prev`
            # are issued one group late so PE never waits on the DVE chain
            prev = None  # (global_group_idx, p_tile)

            def flush_prev():
                nonlocal prev
                if prev is None:
                    return
                pg, pp = prev
                w_ps = psum_w.tile([8, GROUP], f32, tag="w")
                for c in range(CH):
                    nc.tensor.matmul(
                        w_ps[:],
                        mask4[:, 8 * c : 8 * c + 8],
                        pp[:, c, :],
                        start=(c == 0),
                        stop=(c == CH - 1),
                        skip_group_check=True,
                    )
                nc.scalar.activation(
                    w_all[:, pg * GROUP : (pg + 1) * GROUP], w_ps[:], AF_FN
                )
                prev = None

            pending = window_calls(0) + window_calls(1)
            for call in pending[:4]:
                one_gather(*call)
            pending = pending[4:]

            for gg in range(NGROUP):
                w = (gg * GROUP) // WINDOW
                s = gg * GROUP - w * WINDOW   # edge offset within window
                if gg % 4 == 0 and w + 2 < len(wins):
                    pending.extend(window_calls(w + 2))
                if pending:
                    one_gather(*pending.pop(0))
                xiT, xjT, dkT = wtiles[w]
                p_sb = pgrp.tile([128, CH, GROUP], f16, tag="p")
                for c in range(CH):
                    cs = slice(c * 128, (c + 1) * 128)
                    q_ps = psum_q.tile([128, GROUP], f32, tag="q")
                    k_ps = psum_k.tile([128, GROUP], f32, tag="k")
                    nc.tensor.matmul(
                        q_ps[:], wqk[0:64, cs], xiT[0:64, 0, s : s + GROUP]
                    )
                    nc.tensor.matmul(
                        k_ps[:], wqk[64:128, cs], xjT[64:128, 0, s : s + GROUP]
                    )
                    if with_qk_bias:
                        nc.vector.tensor_scalar_add(
                            q_ps[:], q_ps[:], bqk[:, c : c + 1]
                        )
                        nc.vector.tensor_scalar_add(
                            k_ps[:], k_ps[:], bqk[:, 4 + c : 5 + c]
                        )
                    kc_sb = work.tile([128, GROUP], f16, tag="kc")
                    z_sb = work.tile([128, GROUP], f16, tag="z")
                    nc.scalar.copy(kc_sb[:], k_ps[:])
                    nc.vector.tensor_mul(z_sb[:], q_ps[:], kc_sb[:])
                    p_eng = nc.gpsimd if c in POOL_P[gg % 2] else nc.vector
                    p_eng.tensor_mul(
                        p_sb[:, c, :], z_sb[:], dkT[:, c, s : s + GROUP]
                    )
                flush_prev()
                prev = (gg, p_sb)
            flush_prev()

            nc.sync.dma_start(wout_d[:], w_all[:])

    nc.compile()
    _PROGRAM_CACHE[key] = nc
    return nc


def _prep_inputs(dist, nbrs, x_i, W_q, b_q, W_k, b_k, W_dk, b_dk):
    f16 = np.float16
    # x table: fp16 rows [x | x] (256B transpose-gather granularity)
    xtab = np.zeros((NODE_PAD, 128), f16)
    xh = x_i.astype(f16)
    xtab[:N_NODES, :64] = xh
    xtab[:N_NODES, 64:] = xh

    # dk table over NBINS distance bins: silu(W_dk @ feats + b_dk), [h, f]
    # flattened per row (chunk c = heads 2c, 2c+1 at bytes [c*256, (c+1)*256))
    hbin = (CUTOFF - 0.5) / (NBINS - 1)
    dgrid = 0.5 + hbin * np.arange(NBINS)
    fg = _feats_of(dgrid)  # [NBINS, N_RBF] float64
    dkpre = np.einsum("br,hfr->bhf", fg, W_dk.astype(np.float64))
    dkpre += b_dk.astype(np.float64)[None]
    dktab = np.ascontiguousarray(
        _silu(dkpre).reshape(NBINS, HEADS * FEAT).astype(f16)
    )

    # weights in lhsT layout [f_in, h*64+g]
    wqk = np.zeros((128, 512), f16)
    wqk[:64] = W_q.transpose(2, 0, 1).reshape(64, 512).astype(f16)
    wqk[64:] = W_k.transpose(2, 0, 1).reshape(64, 512).astype(f16)

    # head-reduction masks: chunk c covers heads 2c (rows 0-63), 2c+1 (64-127)
    mask4 = np.zeros((128, 32), f16)
    for c in range(CH):
        mask4[0:64, 8 * c + 2 * c] = 1.0
        mask4[64:128, 8 * c + 2 * c + 1] = 1.0

    with_qk_bias = bool(np.any(b_q) or np.any(b_k))
    bqk = None
    if with_qk_bias:
        bqk = np.zeros((128, 8), np.float32)
        for c in range(CH):
            bqk[0:64, c] = b_q[2 * c]
            bqk[64:128, c] = b_q[2 * c + 1]
            bqk[0:64, 4 + c] = b_k[2 * c]
            bqk[64:128, 4 + c] = b_k[2 * c + 1]

    bins_all = np.clip(np.round((dist - 0.5) / hbin), 0, NBINS - 1).astype(np.int64)

    in_maps = []
    for c in range(N_CORES):
        lo = c * E_BASE
        ii = np.zeros(EC, np.int64)
        jj = np.zeros(EC, np.int64)
        bb = np.zeros(EC, np.int64)
        ii[:E_BASE] = nbrs[lo : lo + E_BASE, 0]
        jj[:E_BASE] = nbrs[lo : lo + E_BASE, 1]
        bb[:E_BASE] = bins_all[lo : lo + E_BASE]
        m = {
            "xtab": xtab,
            "dktab": dktab,
            "wqk": wqk,
            "mask4": mask4,
            "idxi": _wrap_idx(ii),
            "idxj": _wrap_idx(jj),
            "idxb": _wrap_idx(bb),
        }
        if with_qk_bias:
            m["bqk"] = bqk
        in_maps.append(m)
    return in_maps, with_qk_bias


def kernel(dist, nbrs, x_i, W_q, b_q, W_k, b_k, W_dk, b_dk):
    from concourse.bass_utils import run_bass_kernel_spmd

    in_maps, with_qk_bias = _prep_inputs(
        np.asarray(dist), np.asarray(nbrs), np.asarray(x_i),
        np.asarray(W_q), np.asarray(b_q), np.asarray(W_k), np.asarray(b_k),
        np.asarray(W_dk), np.asarray(b_dk),
    )
    nc = _build_program(with_qk_bias)
    res = run_bass_kernel_spmd(nc, in_maps, list(range(N_CORES))).results

    out = np.empty((N_EDGES, HEADS), np.float32)
    for c in range(N_CORES):
        w = res[c]["wout"]  # [8, EC] fp16
        out[c * E_BASE : (c + 1) * E_BASE] = w[:, :E_BASE].T.astype(np.float32)
    return out


# revision 12
# speedup vs baseline: 1.5877x; 1.1249x over previous
"""Trainium2 Bass kernel for nn_AttentionHeads (PaiNN-style GNN edge attention).

Computes, per edge e with endpoints (i, j) = nbrs[e]:
    q = W_q @ x_i[i]; k = W_k @ x_i[j]           (per-head linears)
    dk = silu(W_dk @ feats(dist[e]))              (RBF * cosine envelope)
    weights[e, h] = silu(sum_f q*k*dk)

Strategy (8 NeuronCores, data-parallel over edges):
  - All per-edge operands are materialized host-side in the transposed
    layout the TensorEngine wants: an x stream [128, E] (xi features on
    partitions 0-63, xj on 64-127) and a dk stream [128, 4, E] holding
    silu(W_dk @ feats + b_dk) -- a pure function of the binned distance --
    from a 16384-bin table.  The device then streams both with big
    contiguous DMA loads (2048-edge windows, double-buffered three deep);
    no dma_gather, no index tables, and the Pool engine is left free.
  - Per 512-edge group, 4 channel chunks (2 heads each), processed as two
    chunk-pairs: q matmuls write a [128, 1024] PSUM pair tile, k per chunk
    into its own PSUM bank and drained to fp16 SBUF (ACT, or DVE on
    alternate groups -- HW allows only one PSUM operand per vector op),
    z = q*kc as one [128, 1024] DVE multiply per pair, p = z*dk per chunk
    on Pool (scalar_tensor_tensor) or DVE.
  - Head-reduction mask matmuls + final silu run two groups at a time, one
    group behind compute, so no engine waits on the elementwise chain.
"""

import numpy as np

N_NODES = 20000
N_EDGES = 150000
FEAT = 64
HEADS = 8
N_RBF = 20
CUTOFF = 5.0

N_CORES = 8
GROUP = 512                    # edges per compute group
NGROUP = 37                    # groups per core
EC = GROUP * NGROUP            # padded edges per core = 18944
E_BASE = N_EDGES // N_CORES    # real edges per core = 18750
NBINS = 16384                  # distance bins for the dk table
CH = 4                         # channel chunks of 128 (= 2 heads each)
WINDOW = 2048                  # edges per streaming window
ACT_FN = "Silu"
# chunk indices whose p-mul runs on the Pool engine, by group parity
POOL_P = ((0, 2), (0, 1, 2))
# chunk indices whose k-drain runs on DVE instead of ACT, by group parity
DVE_KC = ((), ())


def _silu(v):
    return v / (1.0 + np.exp(-v))


def _feats_of(d):
    # [len(d), N_RBF] float64: sin(n*pi*d/cutoff)/d * cosine envelope
    n = np.arange(1, N_RBF + 1, dtype=np.float64)
    s = np.sin(n * np.pi * d[:, None] / CUTOFF) / d[:, None]
    env = np.where(d < CUTOFF, 0.5 * (np.cos(np.pi * d / CUTOFF) + 1.0), 0.0)
    return s * env[:, None]


_PROGRAM_CACHE = {}


def _build_program(with_qk_bias):
    import concourse.tile as tile
    from concourse import bacc, mybir

    key = (bool(with_qk_bias), ACT_FN, EC, POOL_P, DVE_KC)
    if key in _PROGRAM_CACHE:
        return _PROGRAM_CACHE[key]

    f16 = mybir.dt.float16
    f32 = mybir.dt.float32
    AF = mybir.ActivationFunctionType
    AF_FN = getattr(AF, ACT_FN)
    MULT = mybir.AluOpType.mult

    nc = bacc.Bacc("TRN2", target_bir_lowering=False, debug=False)

    exd = nc.dram_tensor("ex", [128, EC], f16, kind="ExternalInput")
    dkd = nc.dram_tensor("dks", [128, CH, EC], f16, kind="ExternalInput")
    wqk_d = nc.dram_tensor("wqk", [128, 512], f16, kind="ExternalInput")
    mask_d = nc.dram_tensor("mask4", [128, 32], f16, kind="ExternalInput")
    if with_qk_bias:
        bqk_d = nc.dram_tensor("bqk", [128, 8], f32, kind="ExternalInput")
    wout_d = nc.dram_tensor("wout", [8, EC], f16, kind="ExternalOutput")

    wins = []
    o = 0
    while o < EC:
        wins.append((o, min(WINDOW, EC - o)))
        o += WINDOW

    with tile.TileContext(nc) as tc:
        with (
            tc.tile_pool(name="tabs", bufs=1) as tabs,
            tc.tile_pool(name="strm", bufs=3) as strm,
            tc.tile_pool(name="work", bufs=2) as work,
            tc.tile_pool(name="pgrp", bufs=3) as pgrp,
            tc.tile_pool(name="outp", bufs=1) as outp,
            tc.tile_pool(name="psum_q", bufs=2, space="PSUM") as psum_q,
            tc.tile_pool(name="psum_k", bufs=2, space="PSUM") as psum_k,
            tc.tile_pool(name="psum_w", bufs=1, space="PSUM") as psum_w,
        ):
            wqk = tabs.tile([128, 512], f16)
            mask4 = tabs.tile([128, 32], f16)
            w_all = outp.tile([8, EC], f16)

            nc.sync.dma_start(wqk[:], wqk_d[:])
            nc.sync.dma_start(mask4[:], mask_d[:])
            if with_qk_bias:
                bqk = tabs.tile([128, 8], f32)
                nc.sync.dma_start(bqk[:], bqk_d[:])

            wtiles = {}

            def load_window(w):
                if w >= len(wins):
                    return
                o0, m = wins[w]
                if m == WINDOW:
                    ex_w = strm.tile([128, WINDOW], f16, tag="ex")
                    dk_w = strm.tile([128, CH, WINDOW], f16, tag="dk")
                else:
                    ex_w = strm.tile([128, m], f16, tag=f"ex{m}")
                    dk_w = strm.tile([128, CH, m], f16, tag=f"dk{m}")
                wtiles[w] = (ex_w, dk_w)
                nc.sync.dma_start(ex_w[:], exd[:, o0 : o0 + m])
                nc.sync.dma_start(dk_w[:], dkd[:, :, o0 : o0 + m])

            # two-group flush: mask matmuls + one batched silu, one group
            # behind compute so PE never waits on the elementwise chain
            pending = []  # [(global_group_idx, p_tile), ...]

            def flush_pending():
                if not pending:
                    return
                n = len(pending)
                w_ps = psum_w.tile([8, 2 * GROUP], f32, tag="w")
                for gi, (g, pp) in enumerate(pending):
                    for c in range(CH):
                        nc.tensor.matmul(
                            w_ps[:, gi * GROUP : (gi + 1) * GROUP],
                            mask4[:, 8 * c : 8 * c + 8],
                            pp[:, c, :],
                            start=(c == 0),
                            stop=(c == CH - 1),
                            skip_group_check=True,
                        )
                g0 = pending[0][0]
                nc.scalar.activation(
                    w_all[:, g0 * GROUP : (g0 + n) * GROUP],
                    w_ps[:, 0 : n * GROUP],
                    AF_FN,
                )
                del pending[:]

            load_window(0)
            load_window(1)

            for gg in range(NGROUP):
                w = (gg * GROUP) // WINDOW
                s = gg * GROUP - w * WINDOW   # edge offset within window
                if gg % 4 == 0:
                    load_window(w + 2)
                if len(pending) == 2:
                    flush_pending()
                ex_w, dk_w = wtiles[w]
                p_sb = pgrp.tile([128, CH, GROUP], f16, tag="p")
                for half in range(2):
                    q_ps = psum_q.tile([128, 2 * GROUP], f32, tag="q")
                    kc_sb = work.tile([128, 2 * GROUP], f16, tag="kc")
                    z_sb = work.tile([128, 2 * GROUP], f16, tag="z")
                    for ci in range(2):
                        c = 2 * half + ci
                        cs = slice(c * 128, (c + 1) * 128)
                        hs = slice(ci * GROUP, (ci + 1) * GROUP)
                        k_ps = psum_k.tile([128, GROUP], f32, tag="k")
                        nc.tensor.matmul(
                            q_ps[:, hs], wqk[0:64, cs], ex_w[0:64, s : s + GROUP]
                        )
                        nc.tensor.matmul(
                            k_ps[:], wqk[64:128, cs], ex_w[64:128, s : s + GROUP]
                        )
                        if with_qk_bias:
                            nc.vector.tensor_scalar_add(
                                q_ps[:, hs], q_ps[:, hs], bqk[:, c : c + 1]
                            )
                            nc.vector.tensor_scalar_add(
                                k_ps[:], k_ps[:], bqk[:, 4 + c : 5 + c]
                            )
                        kc_eng = nc.vector if c in DVE_KC[gg % 2] else nc.scalar
                        if kc_eng is nc.scalar:
                            nc.scalar.copy(kc_sb[:, hs], k_ps[:])
                        else:
                            nc.vector.tensor_copy(kc_sb[:, hs], k_ps[:])
                    nc.vector.tensor_mul(z_sb[:], q_ps[:], kc_sb[:])
                    for ci in range(2):
                        c = 2 * half + ci
                        hs = slice(ci * GROUP, (ci + 1) * GROUP)
                        p_eng = nc.gpsimd if c in POOL_P[gg % 2] else nc.vector
                        p_eng.tensor_mul(
                            p_sb[:, c, :], z_sb[:, hs],
                            dk_w[:, c, s : s + GROUP],
                        )
                pending.append((gg, p_sb))
            flush_pending()

            nc.sync.dma_start(wout_d[:], w_all[:])

    nc.compile()
    _PROGRAM_CACHE[key] = nc
    return nc


def _prep_inputs(dist, nbrs, x_i, W_q, b_q, W_k, b_k, W_dk, b_dk):
    f16 = np.float16
    xh = np.ascontiguousarray(x_i.astype(f16))

    # dk table over NBINS distance bins: silu(W_dk @ feats + b_dk), flat [h*64+f]
    hbin = (CUTOFF - 0.5) / (NBINS - 1)
    dgrid = 0.5 + hbin * np.arange(NBINS)
    fg = _feats_of(dgrid)  # [NBINS, N_RBF] float64
    dkpre = np.einsum("br,hfr->bhf", fg, W_dk.astype(np.float64))
    dkpre += b_dk.astype(np.float64)[None]
    dktab = _silu(dkpre).reshape(NBINS, HEADS * FEAT).astype(f16)

    # weights in lhsT layout [f_in, h*64+g]
    wqk = np.zeros((128, 512), f16)
    wqk[:64] = W_q.transpose(2, 0, 1).reshape(64, 512).astype(f16)
    wqk[64:] = W_k.transpose(2, 0, 1).reshape(64, 512).astype(f16)

    # head-reduction masks: chunk c covers heads 2c (rows 0-63), 2c+1 (64-127)
    mask4 = np.zeros((128, 32), f16)
    for c in range(CH):
        mask4[0:64, 8 * c + 2 * c] = 1.0
        mask4[64:128, 8 * c + 2 * c + 1] = 1.0

    with_qk_bias = bool(np.any(b_q) or np.any(b_k))
    bqk = None
    if with_qk_bias:
        bqk = np.zeros((128, 8), np.float32)
        for c in range(CH):
            bqk[0:64, c] = b_q[2 * c]
            bqk[64:128, c] = b_q[2 * c + 1]
            bqk[0:64, 4 + c] = b_k[2 * c]
            bqk[64:128, 4 + c] = b_k[2 * c + 1]

    bins_all = np.clip(np.round((dist - 0.5) / hbin), 0, NBINS - 1).astype(np.int64)

    in_maps = []
    for c in range(N_CORES):
        lo = c * E_BASE
        # x stream [128, EC]: xi features on partitions 0-63, xj on 64-127
        ex = np.zeros((128, EC), f16)
        ex[0:64, :E_BASE] = xh[nbrs[lo : lo + E_BASE, 0]].T
        ex[64:128, :E_BASE] = xh[nbrs[lo : lo + E_BASE, 1]].T
        # dk stream [128, CH, EC]: (p, c, e) = dktab[bin[e], c*128+p]
        dke = dktab[bins_all[lo : lo + E_BASE]]  # [E_BASE, 512]
        dks = np.zeros((128, CH, EC), f16)
        dks[:, :, :E_BASE] = (
            dke.T.reshape(CH, 128, E_BASE).transpose(1, 0, 2)
        )
        m = {
            "ex": ex,
            "dks": dks,
            "wqk": wqk,
            "mask4": mask4,
        }
        if with_qk_bias:
            m["bqk"] = bqk
        in_maps.append(m)
    return in_maps, with_qk_bias


def kernel(dist, nbrs, x_i, W_q, b_q, W_k, b_k, W_dk, b_dk):
    from concourse.bass_utils import run_bass_kernel_spmd

    in_maps, with_qk_bias = _prep_inputs(
        np.asarray(dist), np.asarray(nbrs), np.asarray(x_i),
        np.asarray(W_q), np.asarray(b_q), np.asarray(W_k), np.asarray(b_k),
        np.asarray(W_dk), np.asarray(b_dk),
    )
    nc = _build_program(with_qk_bias)
    res = run_bass_kernel_spmd(nc, in_maps, list(range(N_CORES))).results

    out = np.empty((N_EDGES, HEADS), np.float32)
    for c in range(N_CORES):
        w = res[c]["wout"]  # [8, EC] fp16
        out[c * E_BASE : (c + 1) * E_BASE] = w[:, :E_BASE].T.astype(np.float32)
    return out


# revision 14
# speedup vs baseline: 1.5919x; 1.0027x over previous
"""Trainium2 Bass kernel for nn_AttentionHeads (PaiNN-style GNN edge attention).

Computes, per edge e with endpoints (i, j) = nbrs[e]:
    q = W_q @ x_i[i]; k = W_k @ x_i[j]           (per-head linears)
    dk = silu(W_dk @ feats(dist[e]))              (RBF * cosine envelope)
    weights[e, h] = silu(sum_f q*k*dk)

Strategy (8 NeuronCores, data-parallel over edges):
  - All per-edge operands are materialized host-side in the transposed
    layout the TensorEngine wants: an x stream [128, E] (xi features on
    partitions 0-63, xj on 64-127) and a dk stream [128, 4, E] holding
    silu(W_dk @ feats + b_dk) -- a pure function of the binned distance --
    from a 16384-bin table.  The device then streams both with big
    contiguous DMA loads (2048-edge windows, double-buffered three deep);
    no dma_gather, no index tables, and the Pool engine is left free.
  - Per 512-edge group, 4 channel chunks (2 heads each), processed as two
    chunk-pairs: q matmuls write a [128, 1024] PSUM pair tile, k per chunk
    into its own PSUM bank and drained to fp16 SBUF (ACT, or DVE on
    alternate groups -- HW allows only one PSUM operand per vector op),
    z = q*kc as one [128, 1024] DVE multiply per pair, p = z*dk per chunk
    on Pool (scalar_tensor_tensor) or DVE.
  - Head-reduction mask matmuls + final silu run two groups at a time, one
    group behind compute, so no engine waits on the elementwise chain.
"""

import numpy as np

N_NODES = 20000
N_EDGES = 150000
FEAT = 64
HEADS = 8
N_RBF = 20
CUTOFF = 5.0

N_CORES = 8
GROUP = 512                    # edges per compute group
NGROUP = 37                    # groups per core
EC = GROUP * NGROUP            # padded edges per core = 18944
E_BASE = N_EDGES // N_CORES    # real edges per core = 18750
NBINS = 16384                  # distance bins for the dk table
CH = 4                         # channel chunks of 128 (= 2 heads each)
WINDOW = 2048                  # edges per streaming window
ACT_FN = "Silu"
# chunk indices whose p-mul runs on the Pool engine, by group parity
POOL_P = ((0, 2), (0, 1, 2))
# chunk indices whose k-drain runs on DVE instead of ACT, by group parity
DVE_KC = ((), ())


def _silu(v):
    return v / (1.0 + np.exp(-v))


def _feats_of(d):
    # [len(d), N_RBF] float64: sin(n*pi*d/cutoff)/d * cosine envelope
    n = np.arange(1, N_RBF + 1, dtype=np.float64)
    s = np.sin(n * np.pi * d[:, None] / CUTOFF) / d[:, None]
    env = np.where(d < CUTOFF, 0.5 * (np.cos(np.pi * d / CUTOFF) + 1.0), 0.0)
    return s * env[:, None]


_PROGRAM_CACHE = {}


def _build_program(with_qk_bias):
    import concourse.tile as tile
    from concourse import bacc, mybir

    key = (bool(with_qk_bias), ACT_FN, EC, POOL_P, DVE_KC)
    if key in _PROGRAM_CACHE:
        return _PROGRAM_CACHE[key]

    f16 = mybir.dt.float16
    f32 = mybir.dt.float32
    AF = mybir.ActivationFunctionType
    AF_FN = getattr(AF, ACT_FN)
    MULT = mybir.AluOpType.mult

    nc = bacc.Bacc("TRN2", target_bir_lowering=False, debug=False)

    exd = nc.dram_tensor("ex", [128, EC], f16, kind="ExternalInput")
    dkd = nc.dram_tensor("dks", [128, CH, EC], f16, kind="ExternalInput")
    wqk_d = nc.dram_tensor("wqk", [128, 512], f16, kind="ExternalInput")
    mask_d = nc.dram_tensor("mask4", [128, 32], f16, kind="ExternalInput")
    if with_qk_bias:
        bqk_d = nc.dram_tensor("bqk", [128, 8], f32, kind="ExternalInput")
    wout_d = nc.dram_tensor("wout", [8, EC], f16, kind="ExternalOutput")

    wins = []
    o = 0
    while o < EC:
        wins.append((o, min(WINDOW, EC - o)))
        o += WINDOW

    with tile.TileContext(nc) as tc:
        with (
            tc.tile_pool(name="tabs", bufs=1) as tabs,
            tc.tile_pool(name="strm", bufs=3) as strm,
            tc.tile_pool(name="work", bufs=2) as work,
            tc.tile_pool(name="pgrp", bufs=3) as pgrp,
            tc.tile_pool(name="outp", bufs=1) as outp,
            tc.tile_pool(name="psum_q", bufs=2, space="PSUM") as psum_q,
            tc.tile_pool(name="psum_k", bufs=2, space="PSUM") as psum_k,
            tc.tile_pool(name="psum_w", bufs=1, space="PSUM") as psum_w,
        ):
            wqk = tabs.tile([128, 512], f16)
            mask4 = tabs.tile([128, 32], f16)
            w_all = outp.tile([8, EC], f16)

            nc.sync.dma_start(wqk[:], wqk_d[:])
            nc.sync.dma_start(mask4[:], mask_d[:])
            if with_qk_bias:
                bqk = tabs.tile([128, 8], f32)
                nc.sync.dma_start(bqk[:], bqk_d[:])

            wtiles = {}

            def load_window(w):
                if w >= len(wins):
                    return
                o0, m = wins[w]
                if m == WINDOW:
                    ex_w = strm.tile([128, WINDOW], f16, tag="ex")
                    dk_w = strm.tile([128, CH, WINDOW], f16, tag="dk")
                else:
                    ex_w = strm.tile([128, m], f16, tag=f"ex{m}")
                    dk_w = strm.tile([128, CH, m], f16, tag=f"dk{m}")
                wtiles[w] = (ex_w, dk_w)
                nc.sync.dma_start(ex_w[:], exd[:, o0 : o0 + m])
                nc.sync.dma_start(dk_w[:], dkd[:, :, o0 : o0 + m])

            # two-group deferred head reduction: mask matmuls for groups
            # g-2, g-1 are interleaved into group g's matmul stream (after
            # the q matmuls, so they never delay the z chain), the batched
            # silu goes at the END of the body so it never queues ahead of
            # the next group's k-drains on the in-order ACT engine.
            pending = []  # [(global_group_idx, p_tile), ...]

            def w_matmuls(w_ps, gi, pp):
                for c in range(CH):
                    nc.tensor.matmul(
                        w_ps[:, gi * GROUP : (gi + 1) * GROUP],
                        mask4[:, 8 * c : 8 * c + 8],
                        pp[:, c, :],
                        start=(c == 0),
                        stop=(c == CH - 1),
                        skip_group_check=True,
                    )

            def flush_tail():
                # non-interleaved flush for whatever is left at the end
                if not pending:
                    return
                n = len(pending)
                w_ps = psum_w.tile([8, 2 * GROUP], f32, tag="w")
                for gi, (g, pp) in enumerate(pending):
                    w_matmuls(w_ps, gi, pp)
                g0 = pending[0][0]
                nc.scalar.activation(
                    w_all[:, g0 * GROUP : (g0 + n) * GROUP],
                    w_ps[:, 0 : n * GROUP],
                    AF_FN,
                )
                del pending[:]

            load_window(0)
            load_window(1)

            for gg in range(NGROUP):
                w = (gg * GROUP) // WINDOW
                s = gg * GROUP - w * WINDOW   # edge offset within window
                if gg % 4 == 0:
                    load_window(w + 2)
                flush = None
                if len(pending) == 2:
                    w_ps = psum_w.tile([8, 2 * GROUP], f32, tag="w")
                    flush = (w_ps, pending[0], pending[1])
                    pending = []
                ex_w, dk_w = wtiles[w]
                p_sb = pgrp.tile([128, CH, GROUP], f16, tag="p")
                for half in range(2):
                    q_ps = psum_q.tile([128, 2 * GROUP], f32, tag="q")
                    kc_sb = work.tile([128, 2 * GROUP], f16, tag="kc")
                    z_sb = work.tile([128, 2 * GROUP], f16, tag="z")
                    for ci in range(2):
                        c = 2 * half + ci
                        cs = slice(c * 128, (c + 1) * 128)
                        hs = slice(ci * GROUP, (ci + 1) * GROUP)
                        k_ps = psum_k.tile([128, GROUP], f32, tag="k")
                        nc.tensor.matmul(
                            k_ps[:], wqk[64:128, cs], ex_w[64:128, s : s + GROUP]
                        )
                        if with_qk_bias:
                            nc.vector.tensor_scalar_add(
                                k_ps[:], k_ps[:], bqk[:, 4 + c : 5 + c]
                            )
                        nc.scalar.copy(kc_sb[:, hs], k_ps[:])
                    for ci in range(2):
                        c = 2 * half + ci
                        cs = slice(c * 128, (c + 1) * 128)
                        hs = slice(ci * GROUP, (ci + 1) * GROUP)
                        nc.tensor.matmul(
                            q_ps[:, hs], wqk[0:64, cs], ex_w[0:64, s : s + GROUP]
                        )
                        if with_qk_bias:
                            nc.vector.tensor_scalar_add(
                                q_ps[:, hs], q_ps[:, hs], bqk[:, c : c + 1]
                            )
                    if flush is not None:
                        w_matmuls(flush[0], half, flush[1 + half][1])
                    nc.vector.tensor_mul(z_sb[:], q_ps[:], kc_sb[:])
                    for ci in range(2):
                        c = 2 * half + ci
                        hs = slice(ci * GROUP, (ci + 1) * GROUP)
                        p_eng = nc.gpsimd if c in POOL_P[gg % 2] else nc.vector
                        p_eng.tensor_mul(
                            p_sb[:, c, :], z_sb[:, hs],
                            dk_w[:, c, s : s + GROUP],
                        )
                if flush is not None:
                    g0 = flush[1][0]
                    nc.scalar.activation(
                        w_all[:, g0 * GROUP : (g0 + 2) * GROUP],
                        flush[0][:], AF_FN,
                    )
                pending.append((gg, p_sb))
            flush_tail()

            nc.sync.dma_start(wout_d[:], w_all[:])

    nc.compile()
    _PROGRAM_CACHE[key] = nc
    return nc


def _prep_inputs(dist, nbrs, x_i, W_q, b_q, W_k, b_k, W_dk, b_dk):
    f16 = np.float16
    xh = np.ascontiguousarray(x_i.astype(f16))

    # dk table over NBINS distance bins: silu(W_dk @ feats + b_dk), flat [h*64+f]
    hbin = (CUTOFF - 0.5) / (NBINS - 1)
    dgrid = 0.5 + hbin * np.arange(NBINS)
    fg = _feats_of(dgrid)  # [NBINS, N_RBF] float64
    dkpre = np.einsum("br,hfr->bhf", fg, W_dk.astype(np.float64))
    dkpre += b_dk.astype(np.float64)[None]
    dktab = _silu(dkpre).reshape(NBINS, HEADS * FEAT).astype(f16)

    # weights in lhsT layout [f_in, h*64+g]
    wqk = np.zeros((128, 512), f16)
    wqk[:64] = W_q.transpose(2, 0, 1).reshape(64, 512).astype(f16)
    wqk[64:] = W_k.transpose(2, 0, 1).reshape(64, 512).astype(f16)

    # head-reduction masks: chunk c covers heads 2c (rows 0-63), 2c+1 (64-127)
    mask4 = np.zeros((128, 32), f16)
    for c in range(CH):
        mask4[0:64, 8 * c + 2 * c] = 1.0
        mask4[64:128, 8 * c + 2 * c + 1] = 1.0

    with_qk_bias = bool(np.any(b_q) or np.any(b_k))
    bqk = None
    if with_qk_bias:
        bqk = np.zeros((128, 8), np.float32)
        for c in range(CH):
            bqk[0:64, c] = b_q[2 * c]
            bqk[64:128, c] = b_q[2 * c + 1]
            bqk[0:64, 4 + c] = b_k[2 * c]
            bqk[64:128, 4 + c] = b_k[2 * c + 1]

    bins_all = np.clip(np.round((dist - 0.5) / hbin), 0, NBINS - 1).astype(np.int64)

    in_maps = []
    for c in range(N_CORES):
        lo = c * E_BASE
        # x stream [128, EC]: xi features on partitions 0-63, xj on 64-127
        ex = np.zeros((128, EC), f16)
        ex[0:64, :E_BASE] = xh[nbrs[lo : lo + E_BASE, 0]].T
        ex[64:128, :E_BASE] = xh[nbrs[lo : lo + E_BASE, 1]].T
        # dk stream [128, CH, EC]: (p, c, e) = dktab[bin[e], c*128+p]
        dke = dktab[bins_all[lo : lo + E_BASE]]  # [E_BASE, 512]
        dks = np.zeros((128, CH, EC), f16)
        dks[:, :, :E_BASE] = (
            dke.T.reshape(CH, 128, E_BASE).transpose(1, 0, 2)
        )
        m = {
            "ex": ex,
            "dks": dks,
            "wqk": wqk,
            "mask4": mask4,
        }
        if with_qk_bias:
            m["bqk"] = bqk
        in_maps.append(m)
    return in_maps, with_qk_bias


def kernel(dist, nbrs, x_i, W_q, b_q, W_k, b_k, W_dk, b_dk):
    from concourse.bass_utils import run_bass_kernel_spmd

    in_maps, with_qk_bias = _prep_inputs(
        np.asarray(dist), np.asarray(nbrs), np.asarray(x_i),
        np.asarray(W_q), np.asarray(b_q), np.asarray(W_k), np.asarray(b_k),
        np.asarray(W_dk), np.asarray(b_dk),
    )
    nc = _build_program(with_qk_bias)
    res = run_bass_kernel_spmd(nc, in_maps, list(range(N_CORES))).results

    out = np.empty((N_EDGES, HEADS), np.float32)
    for c in range(N_CORES):
        w = res[c]["wout"]  # [8, EC] fp16
        out[c * E_BASE : (c + 1) * E_BASE] = w[:, :E_BASE].T.astype(np.float32)
    return out


# revision 18
# speedup vs baseline: 1.9943x; 1.2528x over previous
"""Trainium2 Bass kernel for nn_AttentionHeads (PaiNN-style GNN edge attention).

Computes, per edge e with endpoints (i, j) = nbrs[e]:
    q = W_q @ x_i[i]; k = W_k @ x_i[j]           (per-head linears)
    dk = silu(W_dk @ feats(dist[e]))              (RBF * cosine envelope)
    weights[e, h] = silu(sum_f q*k*dk)

Strategy (8 NeuronCores, data-parallel over edges):
  - All per-edge operands are materialized host-side in the transposed
    layout the TensorEngine wants: an x stream [128, E] (xi features on
    partitions 0-63, xj on 64-127) and a dk stream [128, 4, E] holding
    silu(W_dk @ feats + b_dk) -- a pure function of the binned distance --
    from a 16384-bin table.  The device then streams both with big
    contiguous DMA loads (2048-edge windows, double-buffered three deep);
    no dma_gather, no index tables, and the Pool engine is left free.
  - Per 512-edge group, 4 channel chunks (2 heads each), processed as two
    chunk-pairs: q matmuls write a [128, 1024] PSUM pair tile, k per chunk
    into its own PSUM bank and drained to fp16 SBUF (ACT, or DVE on
    alternate groups -- HW allows only one PSUM operand per vector op),
    z = q*kc as one [128, 1024] DVE multiply per pair, p = z*dk per chunk
    on Pool (scalar_tensor_tensor) or DVE.
  - Head-reduction mask matmuls + final silu run two groups at a time, one
    group behind compute, so no engine waits on the elementwise chain.
"""

import numpy as np

N_NODES = 20000
N_EDGES = 150000
FEAT = 64
HEADS = 8
N_RBF = 20
CUTOFF = 5.0

N_CORES = 8
GROUP = 512                    # edges per compute group
NGROUP = 37                    # groups per core
EC = GROUP * NGROUP            # padded edges per core = 18944
E_BASE = N_EDGES // N_CORES    # real edges per core = 18750
NBINS = 16384                  # distance bins for the dk table
CH = 4                         # channel chunks of 128 (= 2 heads each)
WINDOW = 2048                  # edges per streaming window
ACT_FN = "Silu"
# chunk indices whose p-mul runs on the Pool engine, by group parity
POOL_P = ((1, 3), (1, 3))


def _silu(v):
    return v / (1.0 + np.exp(-v))


def _feats_of(d):
    # [len(d), N_RBF] float64: sin(n*pi*d/cutoff)/d * cosine envelope
    n = np.arange(1, N_RBF + 1, dtype=np.float64)
    s = np.sin(n * np.pi * d[:, None] / CUTOFF) / d[:, None]
    env = np.where(d < CUTOFF, 0.5 * (np.cos(np.pi * d / CUTOFF) + 1.0), 0.0)
    return s * env[:, None]


_PROGRAM_CACHE = {}


def _build_program(with_qk_bias):
    import concourse.tile as tile
    from concourse import bacc, mybir

    key = (bool(with_qk_bias), ACT_FN, EC, POOL_P)
    if key in _PROGRAM_CACHE:
        return _PROGRAM_CACHE[key]

    f16 = mybir.dt.float16
    f32 = mybir.dt.float32
    AF = mybir.ActivationFunctionType
    AF_FN = getattr(AF, ACT_FN)
    MULT = mybir.AluOpType.mult

    nc = bacc.Bacc("TRN2", target_bir_lowering=False, debug=False)

    exd = nc.dram_tensor("ex", [128, EC], f16, kind="ExternalInput")
    dkd = nc.dram_tensor("dks", [128, CH, EC], f16, kind="ExternalInput")
    wqk_d = nc.dram_tensor("wqk", [128, 512], f16, kind="ExternalInput")
    mask_d = nc.dram_tensor("mask4", [128, 32], f16, kind="ExternalInput")
    if with_qk_bias:
        bqk_d = nc.dram_tensor("bqk", [128, 8], f32, kind="ExternalInput")
    wout_d = nc.dram_tensor("wout", [8, EC], f16, kind="ExternalOutput")

    wins = []
    o = 0
    while o < EC:
        wins.append((o, min(WINDOW, EC - o)))
        o += WINDOW

    with tile.TileContext(nc) as tc:
        with (
            tc.tile_pool(name="tabs", bufs=1) as tabs,
            tc.tile_pool(name="strm", bufs=3) as strm,
            tc.tile_pool(name="work", bufs=3) as work,
            tc.tile_pool(name="pgrp", bufs=3) as pgrp,
            tc.tile_pool(name="outp", bufs=1) as outp,
            tc.tile_pool(name="psum_q", bufs=2, space="PSUM") as psum_q,
            tc.tile_pool(name="psum_k", bufs=3, space="PSUM") as psum_k,
            tc.tile_pool(name="psum_w", bufs=1, space="PSUM") as psum_w,
        ):
            wqk = tabs.tile([128, 512], f16)
            mask4 = tabs.tile([128, 32], f16)
            w_all = outp.tile([8, EC], f16)

            nc.sync.dma_start(wqk[:], wqk_d[:])
            nc.sync.dma_start(mask4[:], mask_d[:])
            if with_qk_bias:
                bqk = tabs.tile([128, 8], f32)
                nc.sync.dma_start(bqk[:], bqk_d[:])

            wtiles = {}

            def load_window(w):
                if w >= len(wins):
                    return
                o0, m = wins[w]
                if m == WINDOW:
                    ex_w = strm.tile([128, WINDOW], f16, tag="ex")
                    dk_w = strm.tile([128, CH, WINDOW], f16, tag="dk")
                else:
                    ex_w = strm.tile([128, m], f16, tag=f"ex{m}")
                    dk_w = strm.tile([128, CH, m], f16, tag=f"dk{m}")
                wtiles[w] = (ex_w, dk_w)
                nc.sync.dma_start(ex_w[:], exd[:, o0 : o0 + m])
                nc.sync.dma_start(dk_w[:], dkd[:, :, o0 : o0 + m])

            # One-group-deferred head reduction: group g-1's mask matmuls
            # are issued after group g's q matmuls (never delaying the z
            # chain), its silu at the end of the body so it never queues
            # ahead of g's k-drains on the in-order ACT engine.  Within a
            # group, all four k matmuls go first (so the ACT drains start
            # as early as possible), then all q matmuls, then both z
            # multiplies, then the p multiplies.
            pending = None  # (global_group_idx, p_tile)

            def flush_prev(prev):
                gp, pp = prev
                w_ps = psum_w.tile([8, GROUP], f32, tag="w")
                for c in range(CH):
                    nc.tensor.matmul(
                        w_ps[:],
                        mask4[:, 8 * c : 8 * c + 8],
                        pp[:, c, :],
                        start=(c == 0),
                        stop=(c == CH - 1),
                        skip_group_check=True,
                    )
                return w_ps

            load_window(0)
            load_window(1)

            for gg in range(NGROUP):
                w = (gg * GROUP) // WINDOW
                s = gg * GROUP - w * WINDOW   # edge offset within window
                if gg % 4 == 0:
                    load_window(w + 2)
                ex_w, dk_w = wtiles[w]
                p_sb = pgrp.tile([128, CH, GROUP], f16, tag="p")
                kc_sb = work.tile([128, CH * GROUP], f16, tag="kc")
                z_sb = work.tile([128, CH * GROUP], f16, tag="z")
                q_tiles = []
                for c in range(CH):
                    cs = slice(c * 128, (c + 1) * 128)
                    k_ps = psum_k.tile([128, GROUP], f32, tag="k")
                    nc.tensor.matmul(
                        k_ps[:], wqk[64:128, cs], ex_w[64:128, s : s + GROUP]
                    )
                    if with_qk_bias:
                        nc.vector.tensor_scalar_add(
                            k_ps[:], k_ps[:], bqk[:, 4 + c : 5 + c]
                        )
                    nc.scalar.copy(
                        kc_sb[:, c * GROUP : (c + 1) * GROUP], k_ps[:]
                    )
                for half in range(2):
                    q_ps = psum_q.tile([128, 2 * GROUP], f32, tag="q")
                    q_tiles.append(q_ps)
                    for ci in range(2):
                        c = 2 * half + ci
                        cs = slice(c * 128, (c + 1) * 128)
                        nc.tensor.matmul(
                            q_ps[:, ci * GROUP : (ci + 1) * GROUP],
                            wqk[0:64, cs],
                            ex_w[0:64, s : s + GROUP],
                        )
                        if with_qk_bias:
                            nc.vector.tensor_scalar_add(
                                q_ps[:, ci * GROUP : (ci + 1) * GROUP],
                                q_ps[:, ci * GROUP : (ci + 1) * GROUP],
                                bqk[:, c : c + 1],
                            )
                w_ps = flush_prev(pending) if pending is not None else None
                for half in range(2):
                    nc.vector.tensor_mul(
                        z_sb[:, half * 2 * GROUP : (half + 1) * 2 * GROUP],
                        q_tiles[half][:],
                        kc_sb[:, half * 2 * GROUP : (half + 1) * 2 * GROUP],
                    )
                for c in range(CH):
                    p_eng = nc.gpsimd if c in POOL_P[gg % 2] else nc.vector
                    p_eng.tensor_mul(
                        p_sb[:, c, :],
                        z_sb[:, c * GROUP : (c + 1) * GROUP],
                        dk_w[:, c, s : s + GROUP],
                    )
                if w_ps is not None:
                    gp = pending[0]
                    nc.scalar.activation(
                        w_all[:, gp * GROUP : (gp + 1) * GROUP], w_ps[:], AF_FN
                    )
                pending = (gg, p_sb)
            w_ps = flush_prev(pending)
            nc.scalar.activation(
                w_all[:, pending[0] * GROUP : (pending[0] + 1) * GROUP],
                w_ps[:], AF_FN,
            )

            nc.sync.dma_start(wout_d[:], w_all[:])

    nc.compile()
    _PROGRAM_CACHE[key] = nc
    return nc


def _prep_inputs(dist, nbrs, x_i, W_q, b_q, W_k, b_k, W_dk, b_dk):
    f16 = np.float16
    xh = np.ascontiguousarray(x_i.astype(f16))

    # dk table over NBINS distance bins: silu(W_dk @ feats + b_dk), flat [h*64+f]
    hbin = (CUTOFF - 0.5) / (NBINS - 1)
    dgrid = 0.5 + hbin * np.arange(NBINS)
    fg = _feats_of(dgrid)  # [NBINS, N_RBF] float64
    dkpre = np.einsum("br,hfr->bhf", fg, W_dk.astype(np.float64))
    dkpre += b_dk.astype(np.float64)[None]
    dktab = _silu(dkpre).reshape(NBINS, HEADS * FEAT).astype(f16)

    # weights in lhsT layout [f_in, h*64+g]
    wqk = np.zeros((128, 512), f16)
    wqk[:64] = W_q.transpose(2, 0, 1).reshape(64, 512).astype(f16)
    wqk[64:] = W_k.transpose(2, 0, 1).reshape(64, 512).astype(f16)

    # head-reduction masks: chunk c covers heads 2c (rows 0-63), 2c+1 (64-127)
    mask4 = np.zeros((128, 32), f16)
    for c in range(CH):
        mask4[0:64, 8 * c + 2 * c] = 1.0
        mask4[64:128, 8 * c + 2 * c + 1] = 1.0

    with_qk_bias = bool(np.any(b_q) or np.any(b_k))
    bqk = None
    if with_qk_bias:
        bqk = np.zeros((128, 8), np.float32)
        for c in range(CH):
            bqk[0:64, c] = b_q[2 * c]
            bqk[64:128, c] = b_q[2 * c + 1]
            bqk[0:64, 4 + c] = b_k[2 * c]
            bqk[64:128, 4 + c] = b_k[2 * c + 1]

    bins_all = np.clip(np.round((dist - 0.5) / hbin), 0, NBINS - 1).astype(np.int64)

    in_maps = []
    for c in range(N_CORES):
        lo = c * E_BASE
        # x stream [128, EC]: xi features on partitions 0-63, xj on 64-127
        ex = np.zeros((128, EC), f16)
        ex[0:64, :E_BASE] = xh[nbrs[lo : lo + E_BASE, 0]].T
        ex[64:128, :E_BASE] = xh[nbrs[lo : lo + E_BASE, 1]].T
        # dk stream [128, CH, EC]: (p, c, e) = dktab[bin[e], c*128+p]
        dke = dktab[bins_all[lo : lo + E_BASE]]  # [E_BASE, 512]
        dks = np.zeros((128, CH, EC), f16)
        dks[:, :, :E_BASE] = (
            dke.T.reshape(CH, 128, E_BASE).transpose(1, 0, 2)
        )
        m = {
            "ex": ex,
            "dks": dks,
            "wqk": wqk,
            "mask4": mask4,
        }
        if with_qk_bias:
            m["bqk"] = bqk
        in_maps.append(m)
    return in_maps, with_qk_bias


def kernel(dist, nbrs, x_i, W_q, b_q, W_k, b_k, W_dk, b_dk):
    from concourse.bass_utils import run_bass_kernel_spmd

    in_maps, with_qk_bias = _prep_inputs(
        np.asarray(dist), np.asarray(nbrs), np.asarray(x_i),
        np.asarray(W_q), np.asarray(b_q), np.asarray(W_k), np.asarray(b_k),
        np.asarray(W_dk), np.asarray(b_dk),
    )
    nc = _build_program(with_qk_bias)
    res = run_bass_kernel_spmd(nc, in_maps, list(range(N_CORES))).results

    out = np.empty((N_EDGES, HEADS), np.float32)
    for c in range(N_CORES):
        w = res[c]["wout"]  # [8, EC] fp16
        out[c * E_BASE : (c + 1) * E_BASE] = w[:, :E_BASE].T.astype(np.float32)
    return out
